# revision 1
# baseline (speedup 1.0000x reference)
"""GNN message passing (src_mul_edge + segment_sum) on 8 Trainium2 cores. v5.

out[n] = sum_{e : dst[e]==n} e_att[e] * src_emb[src[e]]

Non-transpose pair-token gather (baseline-proven DMA mode, tight layout):
  * src_emb rows cast to fp16 (unpadded); consecutive row PAIRS form 256-byte
    tokens in DRAM ([25088, 128] fp16). Token ids fit int16 -> SINGLE index
    window, no lo/hi split. Edge with src row r uses half r%2 of token r//2;
    the unused half is zeroed by the att table.
  * Nodes sorted by total degree, dealt 128 at a time into tiles (lane =
    node); tile span S = max degree in the 128-node window (~1% padding).
    Slot (lane, s) = flat gather position s*128+lane; per-edge descriptors.
  * dma_gather(transpose=False) from DRAM: msg[lane, s, 0:128] = token fp16.
  * att3 [128, S_total, 2] fp16 (host-built, half-selected att or 0) loaded
    once; one broadcast multiply per chunk ([128, S, 2] -> [128, S, 2, 64]);
    strided tensor_reduce over (2S) per tile -> out [128 nodes, 64] fp32,
    one DMA per tile straight to DRAM.
  * Within a node, edges sorted by token id (HBM row locality).
"""

import numpy as np

N_SRC = 50000
N_DST = 50000
D = 64
N_CORES = 8
P = 128
NPAIR = 25088
GCHUNK = 4096             # slots per gather instruction (mult of 128)

_cache: dict = {}

TRACE = False
TRACE_DIR = None
LAST_EXEC_NS = None


def _wrap_idx(idx_flat):
    w = idx_flat.reshape(-1, 16).T
    return np.tile(w, (8, 1))


def _plan(dst_idx, tok, half, att):
    """Single layout over all edges. Returns schedule + per-core arrays."""
    deg = np.bincount(dst_idx, minlength=N_DST)
    nz = np.flatnonzero(deg)
    order = nz[np.argsort(deg[nz], kind="stable")]
    n_nz = len(order)
    npad = (-n_nz) % (P * N_CORES)
    node_seq = np.concatenate([np.full(npad, -1, dtype=np.int64), order])
    n_tiles = len(node_seq) // (P * N_CORES)
    # tile t, core c, lane l -> node_seq[((t*NC)+c)*P + l]
    node_at = node_seq.reshape(n_tiles, N_CORES, P)
    degs = np.where(node_at >= 0, deg[np.clip(node_at, 0, None)], 0)
    S = degs.max(axis=(1, 2)).astype(np.int64)  # per-tile span
    S = np.maximum(S, 1)

    csum = np.concatenate([[0], np.cumsum(S)])
    C = int(csum[-1])  # total slots per lane... columns = slots per lane

    # chunks of tiles with <= budget slots (small priming chunks first)
    chunks = []  # (tile0, ntiles, [S...])
    t0 = 0
    while t0 < n_tiles:
        budget = 1024 if len(chunks) < 4 else GCHUNK
        t1 = t0
        acc = 0
        while t1 < n_tiles and (acc + S[t1]) * P <= budget:
            acc += S[t1]
            t1 += 1
        t1 = max(t1, t0 + 1)
        chunks.append((t0, t1 - t0, tuple(int(x) for x in S[t0:t1])))
        t0 = t1

    # per-edge placement
    ord_of = np.full(N_DST, -1, dtype=np.int64)
    core_of = np.full(N_DST, -1, dtype=np.int64)
    lane_of = np.full(N_DST, -1, dtype=np.int64)
    valid = node_at >= 0
    t_idx = np.broadcast_to(np.arange(n_tiles)[:, None, None], node_at.shape)
    c_idx = np.broadcast_to(np.arange(N_CORES)[None, :, None], node_at.shape)
    l_idx = np.broadcast_to(np.arange(P)[None, None, :], node_at.shape)
    ord_of[node_at[valid]] = t_idx[valid]
    core_of[node_at[valid]] = c_idx[valid]
    lane_of[node_at[valid]] = l_idx[valid]

    eorder = np.lexsort((tok, dst_idx))
    d_sorted = dst_idx[eorder]
    starts = np.concatenate([[0], np.cumsum(deg)])
    rank_e = np.arange(len(dst_idx)) - starts[d_sorted]

    t_e = ord_of[d_sorted]
    c_e = core_of[d_sorted]
    l_e = lane_of[d_sorted]
    s_e = csum[t_e] + rank_e  # slot row within the lane

    idx3 = np.zeros((N_CORES, C, P), dtype=np.int16)
    att4 = np.zeros((N_CORES, C, P, 2), dtype=np.float16)
    idx3[c_e, s_e, l_e] = tok[eorder]
    att4[c_e, s_e, l_e, half[eorder]] = att[eorder]

    return {
        "sched": tuple(chunks),
        "S": tuple(int(x) for x in S),
        "csum": csum,
        "n_tiles": n_tiles,
        "C": C,
        "idx3": idx3,
        "att4": att4,
        "node_at": node_at,
    }


def _build_nc2(sched, csum_list, n_tiles, C):
    import concourse.bacc as bacc
    import concourse.mybir as mybir
    from concourse.tile import TileContext
    from concourse.library_config import mlp

    nc = bacc.Bacc(
        "TRN2", target_bir_lowering=False, debug=False, num_swdge_queues=4
    )
    embP = nc.dram_tensor("embP", [NPAIR, P], mybir.dt.float16, kind="ExternalInput")
    idxT = nc.dram_tensor("idxT", [P, C * P // 16], mybir.dt.int16, kind="ExternalInput")
    attX = nc.dram_tensor("attX", [P, C * P], mybir.dt.float16, kind="ExternalInput")
    out = nc.dram_tensor("out", [n_tiles * P, D], mybir.dt.float32, kind="ExternalOutput")

    with TileContext(nc) as tc:
        nc.gpsimd.load_library(mlp)
        with (
            tc.tile_pool(name="tbl", bufs=1) as tbl,
            tc.tile_pool(name="msg", bufs=8) as msgp,
            tc.tile_pool(name="attx", bufs=3) as attxp,
            tc.tile_pool(name="stg", bufs=3) as stgp,
        ):
            # two-stage idx load: a small head slice unblocks the first
            # gathers while the bulk loads behind it
            head_chunks = min(6, len(sched))
            head_slots = sum(sum(Ss) for _, _, Ss in sched[:head_chunks])
            head_cols = head_slots * P // 16
            tail_cols = C * P // 16 - head_cols
            idx_a = tbl.tile([P, head_cols], mybir.dt.int16, tag="idxa")
            nc.sync.dma_start(idx_a[:], idxT[:, :head_cols])
            if tail_cols > 0:
                idx_b = tbl.tile([P, tail_cols], mybir.dt.int16, tag="idxb")
                nc.sync.dma_start(idx_b[:], idxT[:, head_cols:])

            smax = max(sum(Ss) for _, _, Ss in sched)
            # issue order: keep the priming chunks first, then interleave
            # DVE-heavy (many tiles, small S) and DVE-light chunks so the
            # vector-engine load per gather stays uniform over time
            prime_n = min(4, len(sched))
            rest = sorted(range(prime_n, len(sched)), key=lambda i: -sched[i][1])
            order = list(range(prime_n))
            lo, hi = 0, len(rest) - 1
            while lo <= hi:
                order.append(rest[lo])
                if lo != hi:
                    order.append(rest[hi])
                lo += 1
                hi -= 1
            # greedy queue balance by descriptor count
            qload = [0, 0, 0, 0]
            for ci in order:
                t0, ntl, Ss = sched[ci]
                ssum = sum(Ss)
                nidx = ssum * P
                col0 = csum_list[t0]  # slot offset
                q = min(range(4), key=lambda i: qload[i])
                qload[q] += nidx
                c_lo = col0 * P // 16
                c_hi = (col0 + ssum) * P // 16
                if c_hi <= head_cols:
                    iap = idx_a[:, c_lo:c_hi]
                else:
                    iap = idx_b[:, c_lo - head_cols : c_hi - head_cols]
                msg = msgp.tile([P, smax, P], mybir.dt.float16, tag="m")
                nc.gpsimd.dma_gather(
                    msg[:, :ssum, :], embP[:, :],
                    iap,
                    nidx, nidx, P,
                    transpose=False, single_packet=False, queue_num=q,
                )
                att_x = attxp.tile([P, smax, P], mybir.dt.float16, tag="ax")
                nc.scalar.dma_start(
                    att_x[:, :ssum, :],
                    attX[:, col0 * P : (col0 + ssum) * P]
                    .rearrange("p (s d) -> p s d", d=P),
                )
                nc.vector.tensor_tensor(
                    msg[:, :ssum, :],
                    msg[:, :ssum, :],
                    att_x[:, :ssum, :],
                    mybir.AluOpType.mult,
                )
                so = 0
                for k, S in enumerate(Ss):
                    # pairwise tree over slots with contiguous fp16 adds
                    n = S
                    while n > 1:
                        h_n = n // 2
                        nc.vector.tensor_tensor(
                            msg[:, so : so + h_n, :],
                            msg[:, so : so + h_n, :],
                            msg[:, so + n - h_n : so + n, :],
                            mybir.AluOpType.add,
                        )
                        n = n - h_n
                    stage = stgp.tile([P, D], mybir.dt.float32, tag="st")
                    nc.vector.tensor_tensor(
                        stage[:, :].unsqueeze(1),
                        msg[:, so : so + 1, 0:D],
                        msg[:, so : so + 1, D : 2 * D],
                        mybir.AluOpType.add,
                    )
                    nc.sync.dma_start(
                        out[(t0 + k) * P : (t0 + k + 1) * P, :], stage[:, :]
                    )
                    so += S
    nc.compile()
    return nc


def plan_and_build(src_idx, dst_idx, e_att):
    src_idx = np.asarray(src_idx, dtype=np.int64)
    dst_idx = np.asarray(dst_idx, dtype=np.int64)
    att_flat = np.asarray(e_att, dtype=np.float16).reshape(-1)
    tok = (src_idx // 2).astype(np.int16)
    half = (src_idx & 1).astype(np.int64)
    return _plan(dst_idx, tok, half, att_flat)


def kernel(src_emb, e_att, src_idx, dst_idx):
    from concourse.bass_utils import run_bass_kernel_spmd

    src_emb = np.asarray(src_emb, dtype=np.float32)
    pl = plan_and_build(src_idx, dst_idx, e_att)

    key = (pl["sched"],)
    if key not in _cache:
        _cache.clear()
        _cache[key] = _build_nc2(pl["sched"], pl["csum"], pl["n_tiles"], pl["C"])
    nc = _cache[key]

    embP = np.zeros((NPAIR * 2, D), dtype=np.float16)
    embP[:N_SRC] = src_emb.astype(np.float16)
    embP = np.ascontiguousarray(embP.reshape(NPAIR, P))

    in_maps = []
    for c in range(N_CORES):
        idx_flat = pl["idx3"][c].reshape(-1)  # [C*P] in (slot, lane) order
        # expand att to per-element fp16 [lane, slot*128] (contiguous multiply)
        attx = np.ascontiguousarray(
            np.repeat(pl["att4"][c], D, axis=-1)
            .transpose(1, 0, 2)
            .reshape(P, pl["C"] * P)
        )
        in_maps.append(
            {
                "embP": embP,
                "idxT": np.ascontiguousarray(_wrap_idx(idx_flat)),
                "attX": attx,
            }
        )
    kwargs = {}
    if TRACE:
        kwargs = {"trace": True, "tmpdir": TRACE_DIR}
    res = run_bass_kernel_spmd(nc, in_maps, core_ids=list(range(N_CORES)), **kwargs)
    global LAST_EXEC_NS
    LAST_EXEC_NS = res.exec_time_ns

    out_full = np.zeros((N_DST, D), dtype=np.float32)
    node_at = pl["node_at"]  # [n_tiles, 8, P]
    for c in range(N_CORES):
        ids = node_at[:, c, :].reshape(-1)
        valid = ids >= 0
        out_full[ids[valid]] = res.results[c]["out"][valid]
    return out_full



# revision 4
# speedup vs baseline: 1.2431x; 1.2431x over previous
"""GNN message passing (src_mul_edge + segment_sum) on 8 Trainium2 cores. v6.

out[n] = sum_{e : dst[e]==n} e_att[e] * src_emb[src[e]]

PE scatter-matmul design:
  * src_emb rows cast to fp16; consecutive row PAIRS form 256-byte tokens in
    DRAM ([25088, 128] fp16); token ids fit int16 (single index window).
  * Nodes bin-packed into GROUPS: <=32 nodes, total degree <= 512 slots
    (4 blocks of 128). Groups uniform across cores -> one shared program.
  * Edge slots: per group, edges sorted by token; pad slots use token 0 with
    A = 0. dma_gather(transpose=False) -> msg[slot%128, block, 0:128] fp16,
    round-robin over all 4 SWDGE queues (8 Q7 descriptor cores).
  * Per 128-slot block: two matmuls on the (otherwise idle) tensor engine:
      psum[32 nodes, 64] += A_ev[128,32].T @ msg[:,b,0:64]
                          + A_od[128,32].T @ msg[:,b,64:128]
    where A_ev/A_od hold att for even/odd-half edges (0 elsewhere) and
    PSUM accumulates over the group's 4 blocks.
  * Supertile = 4 groups = one PSUM tile [128, 64] fp32; DVE evacuates to
    SBUF, single DMA to DRAM out. Host unpermutes rows.
"""

import numpy as np

N_SRC = 50000
N_DST = 50000
D = 64
N_CORES = 8
P = 128
NPAIR = 25088
GROUP_SLOTS = 512          # 4 blocks of 128
GROUP_NODES = 32           # PSUM strip width
CHUNK_BLOCKS = 32          # gather chunk = 32 blocks = 4096 slots
PRIME_BLOCKS = 4           # first 4 chunks are 4 blocks each

_cache: dict = {}

TRACE = False
TRACE_DIR = None
LAST_EXEC_NS = None


def _wrap_idx(idx_flat):
    w = idx_flat.reshape(-1, 16).T
    return np.tile(w, (8, 1))


def _binpack(deg):
    """Pack nodes into groups: <=GROUP_NODES nodes, sum(deg) <= GROUP_SLOTS.
    Picks the available degree closest to the remaining per-node target so the
    degree mix stays balanced through the tail. Returns list of node-id lists."""
    maxd = int(deg.max())
    order = np.argsort(-deg, kind="stable")
    nodes_by_deg = [[] for _ in range(maxd + 1)]
    for n in order:
        d = deg[n]
        if d > 0:
            nodes_by_deg[d].append(int(n))
    ptr = [0] * (maxd + 1)
    avail = [len(nodes_by_deg[d]) - ptr[d] for d in range(maxd + 1)]
    remaining = sum(avail[1:])
    groups = []
    while remaining > 0:
        cap = GROUP_SLOTS
        members = []
        while len(members) < GROUP_NODES and cap > 0:
            tgt = cap / (GROUP_NODES - len(members))
            best = -1
            bestdist = None
            d = min(cap, maxd)
            while d >= 1:
                if avail[d] > 0:
                    dist = abs(d - tgt)
                    if bestdist is None or dist < bestdist:
                        bestdist = dist
                        best = d
                    elif d < tgt and dist > bestdist:
                        break
                d -= 1
            if best < 0:
                break
            members.append(nodes_by_deg[best][ptr[best]])
            ptr[best] += 1
            avail[best] -= 1
            remaining -= 1
            cap -= best
        groups.append(members)
    return groups


def _plan(dst_idx, tok, half, att):
    deg = np.bincount(dst_idx, minlength=N_DST)
    groups = _binpack(deg)
    ng = len(groups)
    # per-core group count: multiple of 3 (supertile = 3 groups / 96 psum rows)
    G = -(-ng // N_CORES)
    G = -(-G // 3) * 3
    ng_pad = G * N_CORES
    NB = G * 4                      # blocks per core
    NS = NB * P                     # slots per core

    # group id (global, 0..ng_pad), col within group for every node
    grp_of = np.full(N_DST, -1, dtype=np.int64)
    col_of = np.full(N_DST, -1, dtype=np.int64)
    node_at = np.full((ng_pad, GROUP_NODES), -1, dtype=np.int64)
    for g, members in enumerate(groups):
        m = np.asarray(members, dtype=np.int64)
        grp_of[m] = g
        col_of[m] = np.arange(len(m))
        node_at[g, : len(m)] = m

    E = len(dst_idx)
    g_e = grp_of[dst_idx]
    assert (g_e >= 0).all()
    eorder = np.lexsort((tok, g_e))
    g_s = g_e[eorder]
    # rank of each edge within its group
    gstart = np.searchsorted(g_s, np.arange(ng + 1))
    rank = np.arange(E) - gstart[g_s]
    slot_global = g_s * GROUP_SLOTS + rank      # 0 .. ng*512
    core_e = slot_global // (G * GROUP_SLOTS)
    slot_e = slot_global % (G * GROUP_SLOTS)

    # per-core tables
    idx2 = np.zeros((N_CORES, NS), dtype=np.int16)           # token per slot
    a3 = np.zeros((N_CORES, NS, 2 * GROUP_NODES), dtype=np.float16)
    col_e = col_of[dst_idx][eorder]
    half_e = half[eorder]
    idx2[core_e, slot_e] = tok[eorder]
    a3[core_e, slot_e, col_e + GROUP_NODES * half_e] = att[eorder]

    # chunk schedule (blocks): 4 priming chunks, then CHUNK_BLOCKS
    chunks = []
    b0 = 0
    while b0 < NB:
        nb = PRIME_BLOCKS if len(chunks) < 4 else CHUNK_BLOCKS
        nb = min(nb, NB - b0)
        chunks.append((b0, nb))
        b0 += nb

    return {
        "NB": NB,
        "G": G,
        "chunks": tuple(chunks),
        "idx2": idx2,
        "a3": a3,
        "node_at": node_at,
        "pad_frac": 1.0 - E / (ng_pad * GROUP_SLOTS),
    }


def _build_nc(NB, chunks):
    import concourse.bacc as bacc
    import concourse.mybir as mybir
    from concourse.tile import TileContext
    from concourse.library_config import mlp

    NS = NB * P
    nsuper = NB // 12

    nc = bacc.Bacc(
        "TRN2", target_bir_lowering=False, debug=False, num_swdge_queues=4
    )
    embP = nc.dram_tensor("embP", [NPAIR, P], mybir.dt.float16, kind="ExternalInput")
    idxT = nc.dram_tensor("idxT", [P, NS // 16], mybir.dt.int16, kind="ExternalInput")
    atab = nc.dram_tensor("atab", [P, NB * 64], mybir.dt.float16, kind="ExternalInput")
    out = nc.dram_tensor("out", [nsuper * 96, D], mybir.dt.float32, kind="ExternalOutput")

    with TileContext(nc) as tc:
        nc.gpsimd.load_library(mlp)
        with (
            tc.tile_pool(name="tbl", bufs=1) as tbl,
            tc.tile_pool(name="msg", bufs=6) as msgp,
            tc.tile_pool(name="apool", bufs=4) as apool,
            tc.tile_pool(name="psum", bufs=8, space="PSUM") as psump,
            tc.tile_pool(name="stg", bufs=4) as stgp,
        ):
            # two-stage idx load: head slice unblocks the first gathers
            head_blocks = sum(nb for _, nb in chunks[:5])
            head_cols = head_blocks * 8
            tail_cols = NS // 16 - head_cols
            idx_a = tbl.tile([P, head_cols], mybir.dt.int16, tag="idxa")
            nc.sync.dma_start(idx_a[:], idxT[:, :head_cols])
            if tail_cols > 0:
                idx_b = tbl.tile([P, tail_cols], mybir.dt.int16, tag="idxb")
                nc.sync.dma_start(idx_b[:], idxT[:, head_cols:])

            psum_tiles = {}
            for ci, (b0, nb) in enumerate(chunks):
                q = ci % 4
                c_lo, c_hi = b0 * 8, (b0 + nb) * 8
                if c_hi <= head_cols:
                    iap = idx_a[:, c_lo:c_hi]
                else:
                    iap = idx_b[:, c_lo - head_cols : c_hi - head_cols]
                nidx = nb * P
                msg = msgp.tile([P, CHUNK_BLOCKS, P], mybir.dt.float16, tag="m")
                nc.gpsimd.dma_gather(
                    msg[:, :nb, :], embP[:, :],
                    iap, nidx, nidx, P,
                    transpose=False, single_packet=False, queue_num=q,
                )
                a_t = apool.tile([P, CHUNK_BLOCKS * 64], mybir.dt.float16, tag="a")
                nc.scalar.dma_start(a_t[:, : nb * 64], atab[:, b0 * 64 : (b0 + nb) * 64])

                for j in range(nb):
                    b = b0 + j
                    st = b // 12
                    gl = (b // 4) % 3            # group within supertile
                    if st not in psum_tiles:
                        psum_tiles[st] = psump.tile(
                            [96, D], mybir.dt.float32, tag="ps", name=f"ps{st}"
                        )
                    ps = psum_tiles[st]
                    first = b % 4 == 0
                    last = b % 4 == 3
                    od = ps[32 * gl : 32 * gl + 32, :]
                    nc.tensor.matmul(
                        od, a_t[:, j * 64 : j * 64 + 32], msg[:, j, 0:D],
                        start=first, stop=False,
                    )
                    nc.tensor.matmul(
                        od, a_t[:, j * 64 + 32 : j * 64 + 64], msg[:, j, D : 2 * D],
                        start=False, stop=last,
                    )
                    if last and gl == 2:
                        stage = stgp.tile([96, D], mybir.dt.float32, tag="st")
                        nc.vector.tensor_copy(stage[:, :], ps[:, :])
                        nc.sync.dma_start(out[st * 96 : (st + 1) * 96, :], stage[:, :])
                        del psum_tiles[st]
    nc.compile()
    return nc


def plan_and_build(src_idx, dst_idx, e_att):
    src_idx = np.asarray(src_idx, dtype=np.int64)
    dst_idx = np.asarray(dst_idx, dtype=np.int64)
    att_flat = np.asarray(e_att, dtype=np.float16).reshape(-1)
    tok = (src_idx // 2).astype(np.int16)
    half = (src_idx & 1).astype(np.int64)
    return _plan(dst_idx, tok, half, att_flat)


def kernel(src_emb, e_att, src_idx, dst_idx):
    from concourse.bass_utils import run_bass_kernel_spmd

    src_emb = np.asarray(src_emb, dtype=np.float32)
    pl = plan_and_build(src_idx, dst_idx, e_att)

    key = (pl["NB"], pl["chunks"])
    if key not in _cache:
        _cache.clear()
        _cache[key] = _build_nc(pl["NB"], pl["chunks"])
    nc = _cache[key]

    embP = np.zeros((NPAIR * 2, D), dtype=np.float16)
    embP[:N_SRC] = src_emb.astype(np.float16)
    embP = np.ascontiguousarray(embP.reshape(NPAIR, P))

    NB = pl["NB"]
    in_maps = []
    for c in range(N_CORES):
        # atab layout: [128 (slot in block), NB*64] fp16
        at = np.ascontiguousarray(
            pl["a3"][c].reshape(NB, P, 64).transpose(1, 0, 2).reshape(P, NB * 64)
        )
        in_maps.append(
            {
                "embP": embP,
                "idxT": np.ascontiguousarray(_wrap_idx(pl["idx2"][c].reshape(-1))),
                "atab": at,
            }
        )
    kwargs = {}
    if TRACE:
        kwargs = {"trace": True, "tmpdir": TRACE_DIR}
    res = run_bass_kernel_spmd(nc, in_maps, core_ids=list(range(N_CORES)), **kwargs)
    global LAST_EXEC_NS
    LAST_EXEC_NS = res.exec_time_ns

    out_full = np.zeros((N_DST, D), dtype=np.float32)
    G = pl["G"]
    node_at = pl["node_at"]  # [ng_pad, 32]
    for c in range(N_CORES):
        ids = node_at[c * G : (c + 1) * G].reshape(-1)   # supertile-row order
        valid = ids >= 0
        out_full[ids[valid]] = res.results[c]["out"][valid]
    return out_full
